# revision 14
# baseline (speedup 1.0000x reference)
"""NT-Xent loss on 8 trn2 cores — v5 (fp8 DoubleRow Gram): symmetric Gram + ReduceScatter, with
all cross-partition data movement done by matmuls (no scatter DMAs).

Coverage (identical to v3): core c computes, per i-tile it, the exp'd Gram
slab over local cols [128*it, 5120); the distance-4 superblock [4096, 5120)
is exp'd with bias -ln2 so its double coverage sums to exactly 1x. Row sums
via ACT accum_out; column sums via ones-matmuls accumulated per 512-col
group in PSUM, staged to SBUF rows, rotated to global slice order with a
host-provided 8x8 permutation matmul, and combined across cores by a
32KB->4KB ReduceScatter whose scatter slice for core c is exactly core c's
global row slab. Norms: ones-colsum of x*x stays in row layout; Ln reads
PSUM directly, Exp(-0.5 ln) gives rinv rows; a K=1 ones matmul broadcasts
rinv to 128 partitions. The final row-sum table is transposed with a PE
transpose matmul instead of a strided DMA.
"""
import copy
import math

import numpy as np
import ml_dtypes

N_CORES = 8
B, D = 4096, 512
N = 2 * B
SLAB = N // N_CORES     # 1024
LC = 5 * SLAB           # 5120 local cols used
KT = D // 128           # 4
IT = 8
TEMP = 0.5
INV_T = 1.0 / TEMP
E_DIAG = math.exp(INV_T)
LN2 = math.log(2.0)

_CACHE = {}


class _patched_act_tables:
    """Context: keep Exp/Ln only in natural_log_exp_and_others so the
    table-load pass never thrashes sets while OUR kernel compiles. Set
    positions (= act_func_set_id) are preserved; restored on exit."""

    def __enter__(self):
        import concourse.bacc as bacc_mod
        import concourse.hw_specs as hw_specs
        from concourse import mybir

        self._mod = bacc_mod
        self._orig = bacc_mod.get_activation_tables
        base = hw_specs.get_activation_tables

        def patched(arch):
            tabs = copy.deepcopy(base(arch))
            exp_t = mybir.ActivationFunctionType.Exp
            ln_t = mybir.ActivationFunctionType.Ln
            if any(
                name == "natural_log_exp_and_others" and exp_t in funcs
                for name, funcs in tabs.items()
            ):
                for name, funcs in tabs.items():
                    if name != "natural_log_exp_and_others":
                        funcs.discard(exp_t)
                        funcs.discard(ln_t)
            return tabs

        bacc_mod.get_activation_tables = patched
        return self

    def __exit__(self, *exc):
        self._mod.get_activation_tables = self._orig
        return False


def _plan_colsums():
    plan = {}
    for k in range(5):
        c0 = k * 1024
        for g2 in range(2):
            g0 = c0 + g2 * 512
            lst = []
            for it in range(IT):
                lo = max(128 * it, c0)
                if lo >= c0 + 1024:
                    continue
                glo = max(g0, lo)
                if k == 0:
                    glo = max(glo, 128 * it + 128)   # skip diagonal tile
                gw = g0 + 512 - glo
                if gw > 0:
                    lst.append((it, glo, gw))
            plan[(k, g2)] = lst
    return plan


_CS_PLAN = _plan_colsums()


def _build_nc(loop_k=None):
    import concourse.tile as tile
    from concourse import bacc, mybir

    F32 = mybir.dt.float32
    F32R = mybir.dt.float32r
    BF16 = mybir.dt.bfloat16
    FP8 = mybir.dt.float8e4
    DR = mybir.MatmulPerfMode.DoubleRow
    EXPF = mybir.ActivationFunctionType.Exp
    LNF = mybir.ActivationFunctionType.Ln
    ADD = mybir.AluOpType.add

    nc = bacc.Bacc("TRN2", target_bir_lowering=False, debug=False)
    xt_d = nc.dram_tensor("xt", [D, LC], BF16, kind="ExternalInput")
    ones_d = nc.dram_tensor("ones", [128, 1], BF16, kind="ExternalInput")
    ones1_d = nc.dram_tensor("ones1", [1, 128], BF16, kind="ExternalInput")
    ident_d = nc.dram_tensor("ident", [128, 128], F32, kind="ExternalInput")
    rotm_d = nc.dram_tensor("rotm", [8, 8], F32, kind="ExternalInput")
    out_d = nc.dram_tensor("out", [8, 2], F32, kind="ExternalOutput")
    xtr = xt_d.rearrange("(kt p) l -> p kt l", p=128)

    with tile.TileContext(nc) as tc:
        with (
            tc.tile_pool(name="big", bufs=1) as big,
            tc.tile_pool(name="sqp", bufs=2) as sqp,
            tc.tile_pool(name="etp", bufs=10) as etp,
            tc.tile_pool(name="misc", bufs=1) as misc,
            tc.tile_pool(name="ccp", bufs=1, space="DRAM") as ccp,
            tc.tile_pool(name="ps_pt", bufs=2, space="PSUM") as ps_pt,
            tc.tile_pool(name="ps_ns", bufs=2, space="PSUM") as ps_ns,
            tc.tile_pool(name="ps_cs", bufs=2, space="PSUM") as ps_cs,
        ):
            h = {}

            def body(_iv=None):
                ones = misc.tile([128, 1], BF16, name="ones")
                nc.sync.dma_start(out=ones, in_=ones_d[:, :])
                ones1 = misc.tile([1, 128], BF16, name="ones1")
                nc.sync.dma_start(out=ones1, in_=ones1_d[:, :])
                ident = misc.tile([128, 128], F32R, name="ident")
                nc.sync.dma_start(out=ident, in_=ident_d[:, :].bitcast(F32R))
                rotm = misc.tile([8, 8], F32R, name="rotm")
                nc.sync.dma_start(out=rotm, in_=rotm_d[:, :].bitcast(F32R))

                bln2 = misc.tile([128, 1], F32, name="bln2")
                nc.vector.memset(bln2, -LN2)
                xf = big.tile([128, KT, LC], BF16, name="xf")
                zt = big.tile([128, KT, LC], FP8, name="zt")
                rb = big.tile([128, LC], BF16, name="rb")
                dacc = misc.tile([128, IT, 5], F32, name="dacc")
                nc.vector.memset(dacc, 0.0)
                cls = misc.tile([8, SLAB], F32, name="cls")
                nc.vector.memset(cls, 0.0)
                cc_in = ccp.tile([8, SLAB], F32, name="cc_in")
                cc_rs = ccp.tile([8, 128], F32, name="cc_rs")

                # ---- load + normalize: all chunks, independent chains ----
                for k in range(5):
                    c0 = k * 1024
                    nc.sync.dma_start(
                        out=xf[:, :, c0 : c0 + 1024],
                        in_=xtr[:, :, c0 : c0 + 1024],
                    )
                    sq = sqp.tile([128, KT, 1024], BF16, tag="sq",
                                  name=f"sq{k}")
                    sq_eng = nc.gpsimd if k in (1, 3) else nc.vector
                    sq_eng.tensor_mul(
                        out=sq, in0=xf[:, :, c0 : c0 + 1024],
                        in1=xf[:, :, c0 : c0 + 1024],
                    )
                    lnrow = sqp.tile([1, 1024], F32, tag="lnrow",
                                     name=f"lnrow{k}")
                    for g2 in range(2):
                        g0 = g2 * 512
                        nsum = ps_ns.tile([128, 512], F32, tag="ns",
                                          name=f"ns{k}_{g2}")[0:1, :]
                        for kt in range(KT):
                            nc.tensor.matmul(
                                nsum, ones, sq[:, kt, g0 : g0 + 512],
                                start=(kt == 0), stop=(kt == KT - 1),
                            )
                        nc.scalar.activation(
                            out=lnrow[:, g0 : g0 + 512], in_=nsum, func=LNF
                        )
                    rrow = sqp.tile([1, 1024], BF16, tag="rrow",
                                    name=f"rrow{k}")
                    nc.scalar.activation(
                        out=rrow, in_=lnrow, func=EXPF, scale=-0.5
                    )
                    # broadcast rinv row to 128 partitions via K=1 matmul
                    for g2 in range(2):
                        rbp = ps_ns.tile([128, 512], F32, tag="ns",
                                         name=f"rbp{k}_{g2}")
                        nc.tensor.matmul(
                            rbp, ones1, rrow[:, g2 * 512 : g2 * 512 + 512],
                            start=True, stop=True,
                        )
                        nc.vector.tensor_copy(
                            out=rb[:, c0 + g2 * 512 : c0 + g2 * 512 + 512],
                            in_=rbp,
                        )
                    for kt in range(KT):
                        zt_eng = nc.gpsimd if kt == 0 else nc.vector
                        zt_eng.tensor_mul(
                            out=zt[:, kt, c0 : c0 + 1024],
                            in0=xf[:, kt, c0 : c0 + 1024],
                            in1=rb[:, c0 : c0 + 1024],
                        )

                # ---- positives (row layout, no transpose) ----
                pr = sqp.tile([128, KT, 1024], BF16, tag="sq", name="posprod")
                nc.vector.tensor_mul(
                    out=pr, in0=zt[:, :, 0:1024], in1=zt[:, :, 4096:5120]
                )
                posrow = misc.tile([1, 1024], F32, name="posrow")
                for g2 in range(2):
                    pp = ps_ns.tile([128, 512], F32, tag="ns",
                                    name=f"pp{g2}")[0:1, :]
                    for kt in range(KT):
                        nc.tensor.matmul(
                            pp, ones, pr[:, kt, g2 * 512 : g2 * 512 + 512],
                            start=(kt == 0), stop=(kt == KT - 1),
                        )
                    nc.vector.tensor_copy(
                        out=posrow[:, g2 * 512 : g2 * 512 + 512], in_=pp
                    )

                # ---- Gram + exp + colsums, chunk by chunk ----
                for k in range(5):
                    c0 = k * 1024
                    csmm = [
                        ps_cs.tile([1, 512], F32, tag="cs", name=f"cs{k}_{g2}")
                        for g2 in range(2)
                    ]
                    ets = {}
                    for it in range(IT):
                        s0 = 128 * it
                        lo = max(s0, c0)
                        if lo >= c0 + 1024:
                            continue
                        w = c0 + 1024 - lo
                        pt = ps_pt.tile([128, 1024], F32, tag="pt",
                                        name=f"pt{k}_{it}")
                        for g in range(KT // 2):
                            for b0 in range(0, w, 512):
                                bw = min(512, w - b0)
                                nc.tensor.matmul(
                                    pt[:, b0 : b0 + bw],
                                    zt[:, 2 * g : 2 * g + 2, s0 : s0 + 128],
                                    zt[:, 2 * g : 2 * g + 2,
                                       lo + b0 : lo + b0 + bw],
                                    start=(g == 0), stop=(g == KT // 2 - 1),
                                    perf_mode=DR,
                                )
                        et = etp.tile([128, 1024], BF16, tag="et",
                                      name=f"et{k}_{it}")
                        kw = {"bias": bln2[:, :]} if k == 4 else {}
                        nc.scalar.activation(
                            out=et[:, 0:w], in_=pt[:, 0:w], func=EXPF,
                            scale=INV_T, accum_out=dacc[:, it, k : k + 1],
                            **kw,
                        )
                        ets[it] = et
                    for it in range(IT):
                        s0 = 128 * it
                        lo = max(s0, c0)
                        if lo >= c0 + 1024:
                            continue
                        et = ets[it]
                        for g2 in range(2):
                            lst = _CS_PLAN[(k, g2)]
                            hit = [e for e in lst if e[0] == it]
                            if not hit:
                                continue
                            _, glo, gw = hit[0]
                            g0 = c0 + g2 * 512
                            nc.tensor.matmul(
                                csmm[g2][:, glo - g0 : glo - g0 + gw],
                                ones,
                                et[:, glo - lo : glo - lo + gw],
                                start=(it == lst[0][0]),
                                stop=(it == lst[-1][0]),
                                skip_group_check=True,
                            )
                    # stash this chunk's colsum groups into cls row k
                    for g2 in range(2):
                        lst = _CS_PLAN[(k, g2)]
                        gmin = min(e[1] for e in lst)
                        g0 = k * 1024 + g2 * 512
                        off = g2 * 512 + (gmin - g0)
                        crow = sqp.tile([1, 512], F32, tag="crow",
                                        name=f"crow{k}_{g2}")
                        gw0 = 512 - (gmin - g0)
                        nc.vector.tensor_copy(
                            out=crow[:, 0:gw0],
                            in_=csmm[g2][:, gmin - g0 : 512],
                        )
                        nc.sync.dma_start(
                            out=cls[k : k + 1, off : off + gw0],
                            in_=crow[:, 0:gw0],
                        )
                h.update(rotm=rotm, ident=ident, dacc=dacc, cls=cls,
                         posrow=posrow, cc_in=cc_in, cc_rs=cc_rs)

            def tail():
                rotm, ident = h["rotm"], h["ident"]
                dacc, cls = h["dacc"], h["cls"]
                posrow = h["posrow"]
                cc_in, cc_rs = h["cc_in"], h["cc_rs"]
                ADD = mybir.AluOpType.add
                # rotate local slices to global order; reduce-scatter
                rot = ps_pt.tile([8, SLAB], F32, tag="pt", name="rot")
                for b0 in range(0, SLAB, 512):
                    nc.tensor.matmul(
                        rot[:, b0 : b0 + 512], rotm,
                        cls.bitcast(F32R)[:, b0 : b0 + 512],
                        start=True, stop=True,
                    )
                rotsb = misc.tile([8, SLAB], F32, name="rotsb")
                nc.vector.tensor_copy(out=rotsb, in_=rot)
                nc.sync.dma_start(out=cc_in[:, :], in_=rotsb)
                nc.gpsimd.collective_compute(
                    "ReduceScatter",
                    ADD,
                    replica_groups=[list(range(N_CORES))],
                    ins=[cc_in[:, :]],
                    outs=[cc_rs[:, :]],
                )
                dcol = misc.tile([8, 128], F32, name="dcol")
                nc.sync.dma_start(out=dcol, in_=cc_rs[:, :])

                # row sums [128, it] -> transpose to [it, 128] via PE
                rsum = misc.tile([128, IT], F32R, name="rsum")
                with nc.allow_low_precision(reason="denom partials f32r"):
                    nc.vector.tensor_reduce(
                        out=rsum, in_=dacc, axis=mybir.AxisListType.X, op=ADD,
                    )
                rsT = ps_pt.tile([8, 128], F32R, tag="pt", name="rsT")
                nc.tensor.matmul(
                    rsT, rsum, ident, start=True, stop=True, is_transpose=True
                )
                den = misc.tile([8, 128], F32, name="den")
                nc.vector.tensor_add(
                    out=den, in0=rsT.bitcast(F32), in1=dcol
                )
                negd = misc.tile([8, 1], F32, name="negd")
                nc.vector.memset(negd, -E_DIAG)
                lnden = misc.tile([8, 128], F32, name="lnden")
                nc.scalar.activation(
                    out=lnden, in_=den, func=LNF, bias=negd[:, :], scale=1.0
                )
                out_sb = misc.tile([8, 2], F32, name="out_sb")
                nc.vector.tensor_reduce(
                    out=out_sb[:, 0:1], in_=lnden, axis=mybir.AxisListType.X,
                    op=ADD,
                )
                psum_r = misc.tile([1, 1], F32, name="psum_r")
                nc.vector.tensor_reduce(
                    out=psum_r, in_=posrow, axis=mybir.AxisListType.X, op=ADD,
                )
                nc.vector.memset(out_sb[:, 1:2], 0.0)
                nc.vector.tensor_copy(out=out_sb[0:1, 1:2], in_=psum_r)
                nc.sync.dma_start(out=out_d[:, :], in_=out_sb)

            if loop_k:
                with tc.For_i(0, loop_k, 1):
                    body()
                tail()
            else:
                body()
                tail()

    with _patched_act_tables():
        nc.compile()
    return nc


def _make_in_maps(x_i, x_j):
    x = np.concatenate(
        [np.asarray(x_i, np.float32), np.asarray(x_j, np.float32)], axis=0
    )
    xt = np.ascontiguousarray(x.T).astype(ml_dtypes.bfloat16)  # [D, N]
    ones = np.ones((128, 1), ml_dtypes.bfloat16)
    ones1 = np.ones((1, 128), ml_dtypes.bfloat16)
    ident = np.eye(128, dtype=np.float32)
    in_maps = []
    for c in range(N_CORES):
        cols = (SLAB * c + np.arange(LC)) % N
        rotm = np.zeros((8, 8), np.float32)
        for kk in range(8):
            rotm[kk, (kk + c) % 8] = 1.0
        in_maps.append(
            {
                "xt": np.ascontiguousarray(xt[:, cols]),
                "ones": ones,
                "ones1": ones1,
                "ident": ident,
                "rotm": rotm,
            }
        )
    return in_maps


def _combine(per_core):
    total = 0.0
    for c in range(N_CORES):
        o = np.asarray(per_core[c]["out"])
        total += float(o[:, 0].sum()) - INV_T * float(o[0, 1])
    return np.asarray(total / N, dtype=np.float32)[()]


class _SpmdRunner:
    """Reusable PJRT runner (mirrors concourse.bass2jax.run_bass_via_pjrt but
    keeps the jitted executable and device-resident inputs across calls)."""

    def __init__(self, nc, n_cores):
        import jax
        from jax.sharding import Mesh, NamedSharding, PartitionSpec

        from concourse import mybir
        from concourse.bass2jax import (
            _bass_exec_p,
            install_neuronx_cc_hook,
            partition_id_tensor,
        )

        try:
            from jax.experimental.shard_map import shard_map
        except ImportError:
            from jax.shard_map import shard_map

        install_neuronx_cc_hook()
        self.jax = jax
        self.n_cores = n_cores
        partition_name = (
            nc.partition_id_tensor.name if nc.partition_id_tensor else None
        )
        in_names, out_names, out_avals, zero_outs = [], [], [], []
        for alloc in nc.m.functions[0].allocations:
            if not isinstance(alloc, mybir.MemoryLocationSet):
                continue
            name = alloc.memorylocations[0].name
            if alloc.kind == "ExternalInput":
                if name != partition_name:
                    in_names.append(name)
            elif alloc.kind == "ExternalOutput":
                shape = tuple(alloc.tensor_shape)
                dtype = mybir.dt.np(alloc.dtype)
                out_names.append(name)
                out_avals.append(jax.core.ShapedArray(shape, dtype))
                zero_outs.append(np.zeros(shape, dtype))
        self.in_names = in_names
        self.out_names = out_names
        self.zero_outs = zero_outs
        n_params = len(in_names)
        all_in = list(in_names) + list(out_names)
        if partition_name is not None:
            all_in.append(partition_name)
        donate = tuple(range(n_params, n_params + len(out_names)))

        def _body(*args):
            operands = list(args)
            if partition_name is not None:
                operands.append(partition_id_tensor())
            return tuple(
                _bass_exec_p.bind(
                    *operands,
                    out_avals=tuple(out_avals),
                    in_names=tuple(all_in),
                    out_names=tuple(out_names),
                    lowering_input_output_aliases=(),
                    sim_require_finite=True,
                    sim_require_nnan=True,
                    nc=nc,
                )
            )

        devices = jax.devices()[:n_cores]
        assert len(devices) == n_cores, (
            f"need {n_cores} neuron cores, found {len(jax.devices())}"
        )
        mesh = Mesh(np.asarray(devices), ("core",))
        n_tot = n_params + len(out_names)
        self.fn = jax.jit(
            shard_map(
                _body,
                mesh=mesh,
                in_specs=(PartitionSpec("core"),) * n_tot,
                out_specs=(PartitionSpec("core"),) * len(out_names),
                check_rep=False,
            ),
            donate_argnums=donate,
            keep_unused=True,
        )
        self.sharding = NamedSharding(mesh, PartitionSpec("core"))

    def put_inputs(self, in_maps):
        return [
            self.jax.device_put(
                np.concatenate([np.asarray(m[n]) for m in in_maps], axis=0),
                self.sharding,
            )
            for n in self.in_names
        ]

    def run(self, dev_in):
        import time

        zouts = [
            self.jax.device_put(
                np.zeros((self.n_cores * z.shape[0], *z.shape[1:]), z.dtype),
                self.sharding,
            )
            for z in self.zero_outs
        ]
        t0 = time.perf_counter()
        outs = self.fn(*dev_in, *zouts)
        for o in outs:
            o.block_until_ready()
        dt = time.perf_counter() - t0
        per_core = [dict() for _ in range(self.n_cores)]
        for i, name in enumerate(self.out_names):
            full = np.asarray(outs[i])
            rows = full.shape[0] // self.n_cores
            for c in range(self.n_cores):
                per_core[c][name] = full[c * rows : (c + 1) * rows]
        return per_core, dt



def kernel(x_i, x_j):
    if "runner4" not in _CACHE:
        nc = _build_nc()
        _CACHE["runner4"] = _SpmdRunner(nc, N_CORES)
    r = _CACHE["runner4"]
    per_core, _ = r.run(r.put_inputs(_make_in_maps(x_i, x_j)))
    return _combine(per_core)
